# revision 2
# baseline (speedup 1.0000x reference)
"""AnemllQATLinear Trainium2 kernel (8 NeuronCores, column-parallel).

y = x @ fake_quant(weight).T + bias + lora_scaling * (x @ lora_A.T) @ lora_B.T

Strategy:
  - Shard out_features (O=4096) across 8 cores (512 each). Replicate x.
  - Host prep: x -> x^T as bf16 [I, N] (shared); per-core weight shard
    transposed [I, 512] f32; scale tensors derived from scale_A@scale_B.
  - Device per core: fake-quantize the weight shard into bf16 wq^T tiles
    (closed-form uniform-LUT quantizer, magic-number rounding), then a
    K-cached tiled matmul y[N, 512] = (x^T).T @ wq^T with fused bias add.
  - LoRA is folded into the weight: W_eff = wq + lora_scaling*(lora_B@lora_A).
  - Host gathers per-core y slices -> full [4, 4096, 4096] f32.
"""
import sys
import types
from contextlib import ExitStack

import numpy as np
import ml_dtypes

import concourse.bass as bass
import concourse.mybir as mybir
import concourse.tile as tile
from concourse import bacc
from concourse.bass_utils import run_bass_kernel_spmd
from concourse.kernels.tile_matmul import (
    composable_matmul_tile_kernel,
    dma_from_dram_kxm,
    dma_from_dram_kxn,
    dma_to_dram_mxn,
    k_pool_min_bufs_for_dim,
)

P = 128
N_CORES = 8
O_FULL = 4096
O_LOC = O_FULL // N_CORES  # 512
I_DIM = 4096               # contraction dim K
B, S = 4, 4096
N_ROWS = B * S             # 16384
GS = 128                   # quant group size (== P, so one k-tile == one group)
G = I_DIM // GS            # 32 groups
EPS = 1e-8
LUT_SIZE = 16
LORA_SCALING = 2.0
MAGIC = 12582912.0         # 1.5 * 2**23: f32 round-to-nearest-int via add/sub
QSTEP = 2.0 / (LUT_SIZE - 1)
HALF_IDX = (LUT_SIZE - 1) / 2.0  # 7.5

F32 = mybir.dt.float32
BF16 = mybir.dt.bfloat16
ALU = mybir.AluOpType


def _install_ntff_hook():
    """Enable trace=True under axon: bass_utils needs antenv.axon_hooks."""
    try:
        import antenv

        if "antenv.axon_hooks" not in sys.modules:
            mod = types.ModuleType("antenv.axon_hooks")
            mod._hook = None
            mod.set_axon_ntff_profile_hook = lambda h: setattr(mod, "_hook", h)
            mod.get_axon_ntff_profile_hook = lambda: mod._hook
            sys.modules["antenv.axon_hooks"] = mod
            antenv.axon_hooks = mod
        from trn_agent_boot.trn_boot import _ntff_profile_via_ctypes

        sys.modules["antenv.axon_hooks"].set_axon_ntff_profile_hook(
            _ntff_profile_via_ctypes("/opt/axon/libaxon_pjrt.so")
        )
        import concourse.bass_utils as bass_utils

        bass_utils.upload_artifacts = lambda tmpdir: str(tmpdir)
    except Exception:
        pass


def build_nc(use_lora: bool):
    nc = bacc.Bacc("TRN2", target_bir_lowering=False, debug=False, num_devices=N_CORES)

    xt = nc.dram_tensor("xt", [I_DIM, N_ROWS], BF16, kind="ExternalInput")
    wt = nc.dram_tensor("wt", [I_DIM, O_LOC], F32, kind="ExternalInput")
    # scale-derived per-group tensors, transposed to [G, O_LOC]:
    #   rs = half_idx / s      (normalize factor; clamp at +-half_idx)
    #   cs = lut_slope * s     (rescale slope)
    rs = nc.dram_tensor("rs", [G, O_LOC], F32, kind="ExternalInput")
    cs = nc.dram_tensor("cs", [G, O_LOC], F32, kind="ExternalInput")
    bias_in = nc.dram_tensor("biasv", [1, O_LOC], F32, kind="ExternalInput")
    if use_lora:
        # aw = lut_intercept * s expanded to [I, O_LOC] + lora fold
        aw = nc.dram_tensor("aw", [I_DIM, O_LOC], F32, kind="ExternalInput")
        asb = None
    else:
        aw = None
        asb = nc.dram_tensor("asb", [G, O_LOC], F32, kind="ExternalInput")
    y = nc.dram_tensor("y", [N_ROWS, O_LOC], F32, kind="ExternalOutput")

    with ExitStack() as ctx:
        tc = ctx.enter_context(tile.TileContext(nc))
        constp = ctx.enter_context(tc.tile_pool(name="const", bufs=1))
        qpool = ctx.enter_context(tc.tile_pool(name="qpool", bufs=3))
        qbc = ctx.enter_context(tc.tile_pool(name="qbc", bufs=3))
        wqd_pool = ctx.enter_context(tc.tile_pool(name="wqd", bufs=1, space="DRAM"))
        kxn_bufs = k_pool_min_bufs_for_dim(I_DIM)
        kxm_pool = ctx.enter_context(tc.tile_pool(name="kxm_pool", bufs=4))
        kxn_pool = ctx.enter_context(tc.tile_pool(name="kxn_pool", bufs=kxn_bufs))

        # bias broadcast to all partitions once
        bias_bc = constp.tile([P, O_LOC], F32)
        nc.sync.dma_start(out=bias_bc[:], in_=bias_in[:].broadcast_to([P, O_LOC]))

        # ---- Phase A: fake-quantize weight shard into wq^T (bf16, [pi, po, f]) ----
        wq_dram = wqd_pool.tile([P, G, O_LOC], BF16)

        for g in range(G):
            wt_t = qpool.tile([P, O_LOC], F32, tag="wt")
            nc.sync.dma_start(out=wt_t[:], in_=wt[g * P:(g + 1) * P, :])
            rB = qbc.tile([P, O_LOC], F32, tag="rB")
            nc.sync.dma_start(out=rB[:], in_=rs[g:g + 1, :].broadcast_to([P, O_LOC]))
            cB = qbc.tile([P, O_LOC], F32, tag="cB")
            nc.sync.dma_start(out=cB[:], in_=cs[g:g + 1, :].broadcast_to([P, O_LOC]))
            addB = qbc.tile([P, O_LOC], F32, tag="addB")
            if use_lora:
                nc.sync.dma_start(out=addB[:], in_=aw[g * P:(g + 1) * P, :])
            else:
                nc.sync.dma_start(
                    out=addB[:], in_=asb[g:g + 1, :].broadcast_to([P, O_LOC])
                )

            u = qpool.tile([P, O_LOC], F32, tag="u")
            # u = w * (half_idx/s)
            nc.vector.tensor_tensor(out=u[:], in0=wt_t[:], in1=rB[:], op=ALU.mult)
            # clamp to [-half_idx, half_idx]
            nc.vector.tensor_scalar(
                out=u[:], in0=u[:], scalar1=HALF_IDX, scalar2=-HALF_IDX,
                op0=ALU.min, op1=ALU.max,
            )
            # t3 = (u + half_idx) + MAGIC  -> MAGIC + round(t), t in [0, 15]
            nc.vector.tensor_scalar(
                out=u[:], in0=u[:], scalar1=HALF_IDX, scalar2=MAGIC,
                op0=ALU.add, op1=ALU.add,
            )
            # p = (t3 - MAGIC) * (lut_slope * s)
            nc.vector.scalar_tensor_tensor(
                out=u[:], in0=u[:], scalar=MAGIC, in1=cB[:],
                op0=ALU.subtract, op1=ALU.mult,
            )
            # wq = p + (lut_intercept * s [+ lora fold])   (cast to bf16)
            wq_sb = qpool.tile([P, O_LOC], BF16, tag="wq")
            nc.vector.tensor_tensor(out=wq_sb[:], in0=u[:], in1=addB[:], op=ALU.add)
            nc.sync.dma_start(out=wq_dram[:, g, :], in_=wq_sb[:])

        # ---- Phase B: y[N, O_LOC] = (x^T).T @ wq^T + bias ----
        kxm_producer, kxm_shape = dma_from_dram_kxm(kxm_pool, xt[:])
        kxn_producer, kxn_shape = dma_from_dram_kxn(kxn_pool, wq_dram[:])
        mxn_consumer = dma_to_dram_mxn(y[:])

        def bias_reducer(nc_, psum, sbuf, md):
            nc_.vector.tensor_tensor(
                out=sbuf[:],
                in0=psum[:, :md.n_slice_size],
                in1=bias_bc[:, :md.n_slice_size],
                op=ALU.add,
            )

        composable_matmul_tile_kernel(
            tc=tc,
            kxm_shape=kxm_shape,
            kxn_shape=kxn_shape,
            output_type=F32,
            kxm_producer=kxm_producer,
            kxn_producer=kxn_producer,
            mxn_consumer=mxn_consumer,
            mxn_subtile_reducer=bias_reducer,
            cache_tiles=True,
            psum_n_bufs=2,
        )

    nc.compile()
    return nc


_NC_CACHE: dict = {}


def _get_nc(use_lora: bool):
    if use_lora not in _NC_CACHE:
        _NC_CACHE[use_lora] = build_nc(use_lora)
    return _NC_CACHE[use_lora]


def kernel(x, weight, bias, scale_A, scale_B, lut, lora_A, lora_B, **_):
    _install_ntff_hook()

    x = np.asarray(x, dtype=np.float32)
    weight = np.asarray(weight, dtype=np.float32)
    bias = np.asarray(bias, dtype=np.float32)
    scale_A = np.asarray(scale_A, dtype=np.float32)
    scale_B = np.asarray(scale_B, dtype=np.float32)
    lut = np.asarray(lut, dtype=np.float32)
    lora_A = np.asarray(lora_A, dtype=np.float32)
    lora_B = np.asarray(lora_B, dtype=np.float32)

    # ---- host prep ----
    s_full = np.maximum(scale_A @ scale_B, EPS)  # [O, G]

    # affine fit of the LUT: lut[k] ~= a + b*k (exact for linspace)
    a_fit = float(lut[0])
    b_fit = float(lut[-1] - lut[0]) / (LUT_SIZE - 1)
    idx = np.arange(LUT_SIZE, dtype=np.float32)
    affine_ok = np.max(np.abs(lut - (a_fit + b_fit * idx))) <= 1e-6 * max(
        1.0, np.max(np.abs(lut))
    )

    wl = None
    use_lora = bool(np.any(lora_B != 0.0)) or not affine_ok
    if use_lora:
        wl = (LORA_SCALING * (lora_B @ lora_A)).astype(np.float32)  # [O, I]

    if not affine_ok:
        # general LUT fallback: quantize on host, ship wq via the lora path
        grouped = weight.reshape(O_FULL, G, GS)
        norm = np.clip(grouped / s_full[:, :, None], -1.0, 1.0)
        qidx = np.clip(
            np.round((norm + 1.0) / QSTEP).astype(np.int32), 0, LUT_SIZE - 1
        )
        wq_host = (lut[qidx] * s_full[:, :, None]).reshape(O_FULL, I_DIM)
        wl = wl + wq_host if wl is not None else wq_host
        # zero the device quantizer output: scales such that result is 0
        s_eff = s_full * 0.0 + 1.0
        rs_full = np.zeros_like(s_full)
        cs_full = np.zeros_like(s_full)
        aw_base = np.zeros_like(s_full)
    else:
        s_eff = s_full
        rs_full = (HALF_IDX / s_eff).astype(np.float32)          # [O, G]
        cs_full = (np.float64(b_fit) * s_eff).astype(np.float32)  # [O, G]
        aw_base = (np.float64(a_fit) * s_eff).astype(np.float32)  # [O, G]

    x2 = x.reshape(N_ROWS, I_DIM)
    xt_bf16 = np.ascontiguousarray(x2.astype(ml_dtypes.bfloat16).T)  # [I, N]

    in_maps = []
    for c in range(N_CORES):
        sl = slice(c * O_LOC, (c + 1) * O_LOC)
        m = {
            "xt": xt_bf16,
            "wt": np.ascontiguousarray(weight[sl].T),          # [I, O_LOC]
            "rs": np.ascontiguousarray(rs_full[sl].T),         # [G, O_LOC]
            "cs": np.ascontiguousarray(cs_full[sl].T),         # [G, O_LOC]
            "biasv": bias[sl].reshape(1, O_LOC).copy(),
        }
        if use_lora:
            # aw[i, o] = a*s[o, g(i)] + wl[o, i]
            aw_full = np.repeat(aw_base[sl], GS, axis=1) + wl[sl]  # [O_LOC, I]
            m["aw"] = np.ascontiguousarray(aw_full.T)              # [I, O_LOC]
        else:
            m["asb"] = np.ascontiguousarray(aw_base[sl].T)         # [G, O_LOC]
        in_maps.append(m)

    nc = _get_nc(use_lora)
    res = run_bass_kernel_spmd(
        nc, in_maps, core_ids=list(range(N_CORES)), trace=False
    )
    global LAST_RESULT
    LAST_RESULT = res

    y = np.concatenate([res.results[c]["y"] for c in range(N_CORES)], axis=1)
    return np.ascontiguousarray(y.reshape(B, S, O_FULL).astype(np.float32))


if __name__ == "__main__":
    rng = np.random.default_rng(0)
    x = rng.standard_normal((B, S, I_DIM), dtype=np.float32)
    weight = (rng.standard_normal((O_FULL, I_DIM), dtype=np.float32) * 0.02)
    bias = rng.uniform(-0.015, 0.015, O_FULL).astype(np.float32)
    sf = np.maximum(np.abs(weight.reshape(O_FULL, G, GS)).max(axis=2), EPS)
    u, s, vh = np.linalg.svd(sf, full_matrices=False)
    scale_A = (u[:, :4] * s[:4]).astype(np.float32)
    scale_B = vh[:4, :].astype(np.float32)
    lut = np.linspace(-1, 1, LUT_SIZE, dtype=np.float32)
    lora_A = rng.standard_normal((16, I_DIM), dtype=np.float32) * 0.02
    lora_B = np.zeros((O_FULL, 16), dtype=np.float32)
    y = kernel(x=x, weight=weight, bias=bias, scale_A=scale_A, scale_B=scale_B,
               lut=lut, lora_A=lora_A, lora_B=lora_B)
    print("kernel output:", y.shape, y.dtype)


# revision 5
# speedup vs baseline: 1.0832x; 1.0832x over previous
"""AnemllQATLinear Trainium2 kernel (8 NeuronCores, column-parallel).

y = x @ fake_quant(weight).T + bias + lora_scaling * (x @ lora_A.T) @ lora_B.T

Strategy:
  - Shard out_features (O=4096) across 8 cores (512 each). Replicate x.
  - Host prep: x -> x^T as bf16 [I, N] (shared); per-core weight shard
    transposed [I, 512] f32; scale tensors derived from scale_A@scale_B.
  - Device per core: fake-quantize the weight shard into bf16 wq^T tiles
    (closed-form uniform-LUT quantizer, magic-number rounding), then a
    K-cached tiled matmul y[N, 512] = (x^T).T @ wq^T with fused bias add.
  - LoRA is folded into the weight: W_eff = wq + lora_scaling*(lora_B@lora_A).
  - Host gathers per-core y slices -> full [4, 4096, 4096] f32.
"""
import sys
import types
from contextlib import ExitStack

import numpy as np
import ml_dtypes

import concourse.bass as bass
import concourse.mybir as mybir
import concourse.tile as tile
from concourse import bacc
from concourse.bass_utils import run_bass_kernel_spmd
from concourse.kernels.tile_matmul import (
    ShapeInfo,
    composable_matmul_tile_kernel,
    dma_from_dram_kxm,
    dma_to_dram_mxn,
)

P = 128
N_CORES = 8
O_FULL = 4096
O_LOC = O_FULL // N_CORES  # 512
I_DIM = 4096               # contraction dim K
B, S = 4, 4096
N_ROWS = B * S             # 16384
GS = 128                   # quant group size (== P, so one k-tile == one group)
G = I_DIM // GS            # 32 groups
EPS = 1e-8
LUT_SIZE = 16
LORA_SCALING = 2.0
MAGIC = 12582912.0         # 1.5 * 2**23: f32 round-to-nearest-int via add/sub
QSTEP = 2.0 / (LUT_SIZE - 1)
HALF_IDX = (LUT_SIZE - 1) / 2.0  # 7.5

F32 = mybir.dt.float32
BF16 = mybir.dt.bfloat16
ALU = mybir.AluOpType


def _install_ntff_hook():
    """Enable trace=True under axon: bass_utils needs antenv.axon_hooks."""
    try:
        import antenv

        if "antenv.axon_hooks" not in sys.modules:
            mod = types.ModuleType("antenv.axon_hooks")
            mod._hook = None
            mod.set_axon_ntff_profile_hook = lambda h: setattr(mod, "_hook", h)
            mod.get_axon_ntff_profile_hook = lambda: mod._hook
            sys.modules["antenv.axon_hooks"] = mod
            antenv.axon_hooks = mod
        from trn_agent_boot.trn_boot import _ntff_profile_via_ctypes

        sys.modules["antenv.axon_hooks"].set_axon_ntff_profile_hook(
            _ntff_profile_via_ctypes("/opt/axon/libaxon_pjrt.so")
        )
        import concourse.bass_utils as bass_utils

        bass_utils.upload_artifacts = lambda tmpdir: str(tmpdir)
    except Exception:
        pass


def build_nc(use_lora: bool):
    nc = bacc.Bacc("TRN2", target_bir_lowering=False, debug=False, num_devices=N_CORES)

    xt = nc.dram_tensor("xt", [I_DIM, N_ROWS], BF16, kind="ExternalInput")
    wt = nc.dram_tensor("wt", [I_DIM, O_LOC], F32, kind="ExternalInput")
    # scale-derived per-group tensors, transposed to [G, O_LOC]:
    #   rs = half_idx / s      (normalize factor; clamp at +-half_idx)
    #   cs = lut_slope * s     (rescale slope)
    rs = nc.dram_tensor("rs", [G, O_LOC], F32, kind="ExternalInput")
    cs = nc.dram_tensor("cs", [G, O_LOC], F32, kind="ExternalInput")
    bias_in = nc.dram_tensor("biasv", [1, O_LOC], F32, kind="ExternalInput")
    if use_lora:
        # aw = lut_intercept * s expanded to [I, O_LOC] + lora fold
        aw = nc.dram_tensor("aw", [I_DIM, O_LOC], F32, kind="ExternalInput")
        asb = None
    else:
        aw = None
        asb = nc.dram_tensor("asb", [G, O_LOC], F32, kind="ExternalInput")
    y = nc.dram_tensor("y", [N_ROWS, O_LOC], F32, kind="ExternalOutput")

    K_TILE = 512
    K_TILES_N = I_DIM // K_TILE  # 8
    K_SUB = K_TILE // P          # 4 groups per k-tile

    with ExitStack() as ctx:
        tc = ctx.enter_context(tile.TileContext(nc))
        constp = ctx.enter_context(tc.tile_pool(name="const", bufs=1))
        qpool = ctx.enter_context(tc.tile_pool(name="qpool", bufs=3))
        qbc = ctx.enter_context(tc.tile_pool(name="qbc", bufs=3))
        wq_pool = ctx.enter_context(tc.tile_pool(name="wq_pool", bufs=1))
        kxm_pool = ctx.enter_context(tc.tile_pool(name="kxm_pool", bufs=4))

        # bias broadcast to all partitions once
        bias_bc = constp.tile([P, O_LOC], F32)
        nc.sync.dma_start(out=bias_bc[:], in_=bias_in[:].broadcast_to([P, O_LOC]))

        # ---- Phase A: fake-quantize weight shard into SBUF-resident wq^T tiles
        # (bf16, [128, K_SUB, O_LOC] per k-tile) that phase B reads directly ----
        wq_tiles = [
            wq_pool.tile([P, K_SUB, O_LOC], BF16, tag=f"wqt{k}", name=f"wqt{k}")
            for k in range(K_TILES_N)
        ]

        for g in range(G):
            wt_t = qpool.tile([P, O_LOC], F32, tag="wt")
            nc.sync.dma_start(out=wt_t[:], in_=wt[g * P:(g + 1) * P, :])
            rB = qbc.tile([P, O_LOC], F32, tag="rB")
            nc.sync.dma_start(out=rB[:], in_=rs[g:g + 1, :].broadcast_to([P, O_LOC]))
            cB = qbc.tile([P, O_LOC], F32, tag="cB")
            nc.sync.dma_start(out=cB[:], in_=cs[g:g + 1, :].broadcast_to([P, O_LOC]))
            addB = qbc.tile([P, O_LOC], F32, tag="addB")
            if use_lora:
                nc.sync.dma_start(out=addB[:], in_=aw[g * P:(g + 1) * P, :])
            else:
                nc.sync.dma_start(
                    out=addB[:], in_=asb[g:g + 1, :].broadcast_to([P, O_LOC])
                )

            u = qpool.tile([P, O_LOC], F32, tag="u")
            # u = w * (half_idx/s)
            nc.vector.tensor_tensor(out=u[:], in0=wt_t[:], in1=rB[:], op=ALU.mult)
            # clamp to [-half_idx, half_idx]
            nc.vector.tensor_scalar(
                out=u[:], in0=u[:], scalar1=HALF_IDX, scalar2=-HALF_IDX,
                op0=ALU.min, op1=ALU.max,
            )
            # t3 = (u + half_idx) + MAGIC  -> MAGIC + round(t), t in [0, 15]
            nc.vector.tensor_scalar(
                out=u[:], in0=u[:], scalar1=HALF_IDX, scalar2=MAGIC,
                op0=ALU.add, op1=ALU.add,
            )
            # p = (t3 - MAGIC) * (lut_slope * s)
            nc.vector.scalar_tensor_tensor(
                out=u[:], in0=u[:], scalar=MAGIC, in1=cB[:],
                op0=ALU.subtract, op1=ALU.mult,
            )
            # wq = p + (lut_intercept * s [+ lora fold])   (cast to bf16)
            nc.vector.tensor_tensor(
                out=wq_tiles[g // K_SUB][:, g % K_SUB, :],
                in0=u[:], in1=addB[:], op=ALU.add,
            )

        # ---- Phase B: y[N, O_LOC] = (x^T).T @ wq^T + bias ----
        kxm_producer, kxm_shape = dma_from_dram_kxm(kxm_pool, xt[:])
        kxn_shape = ShapeInfo(pdims=((P, G),), fdims=(O_LOC,))

        def kxn_producer(nc_, md):
            return wq_tiles[md.k_tile_idx][:]

        mxn_consumer = dma_to_dram_mxn(y[:])

        def bias_reducer(nc_, psum, sbuf, md):
            nc_.vector.tensor_tensor(
                out=sbuf[:],
                in0=psum[:, :md.n_slice_size],
                in1=bias_bc[:, :md.n_slice_size],
                op=ALU.add,
            )

        composable_matmul_tile_kernel(
            tc=tc,
            kxm_shape=kxm_shape,
            kxn_shape=kxn_shape,
            output_type=F32,
            kxm_producer=kxm_producer,
            kxn_producer=kxn_producer,
            mxn_consumer=mxn_consumer,
            mxn_subtile_reducer=bias_reducer,
            cache_tiles=True,
            psum_n_bufs=2,
        )

    nc.compile()
    return nc


_NC_CACHE: dict = {}


def _get_nc(use_lora: bool):
    if use_lora not in _NC_CACHE:
        _NC_CACHE[use_lora] = build_nc(use_lora)
    return _NC_CACHE[use_lora]


def kernel(x, weight, bias, scale_A, scale_B, lut, lora_A, lora_B, **_):
    _install_ntff_hook()

    x = np.asarray(x, dtype=np.float32)
    weight = np.asarray(weight, dtype=np.float32)
    bias = np.asarray(bias, dtype=np.float32)
    scale_A = np.asarray(scale_A, dtype=np.float32)
    scale_B = np.asarray(scale_B, dtype=np.float32)
    lut = np.asarray(lut, dtype=np.float32)
    lora_A = np.asarray(lora_A, dtype=np.float32)
    lora_B = np.asarray(lora_B, dtype=np.float32)

    # ---- host prep ----
    s_full = np.maximum(scale_A @ scale_B, EPS)  # [O, G]

    # affine fit of the LUT: lut[k] ~= a + b*k (exact for linspace)
    a_fit = float(lut[0])
    b_fit = float(lut[-1] - lut[0]) / (LUT_SIZE - 1)
    idx = np.arange(LUT_SIZE, dtype=np.float32)
    affine_ok = np.max(np.abs(lut - (a_fit + b_fit * idx))) <= 1e-6 * max(
        1.0, np.max(np.abs(lut))
    )

    wl = None
    use_lora = bool(np.any(lora_B != 0.0)) or not affine_ok
    if use_lora:
        wl = (LORA_SCALING * (lora_B @ lora_A)).astype(np.float32)  # [O, I]

    if not affine_ok:
        # general LUT fallback: quantize on host, ship wq via the lora path
        grouped = weight.reshape(O_FULL, G, GS)
        norm = np.clip(grouped / s_full[:, :, None], -1.0, 1.0)
        qidx = np.clip(
            np.round((norm + 1.0) / QSTEP).astype(np.int32), 0, LUT_SIZE - 1
        )
        wq_host = (lut[qidx] * s_full[:, :, None]).reshape(O_FULL, I_DIM)
        wl = wl + wq_host if wl is not None else wq_host
        # zero the device quantizer output: scales such that result is 0
        s_eff = s_full * 0.0 + 1.0
        rs_full = np.zeros_like(s_full)
        cs_full = np.zeros_like(s_full)
        aw_base = np.zeros_like(s_full)
    else:
        s_eff = s_full
        rs_full = (HALF_IDX / s_eff).astype(np.float32)          # [O, G]
        cs_full = (np.float64(b_fit) * s_eff).astype(np.float32)  # [O, G]
        aw_base = (np.float64(a_fit) * s_eff).astype(np.float32)  # [O, G]

    x2 = x.reshape(N_ROWS, I_DIM)
    xt_bf16 = np.ascontiguousarray(x2.astype(ml_dtypes.bfloat16).T)  # [I, N]

    in_maps = []
    for c in range(N_CORES):
        sl = slice(c * O_LOC, (c + 1) * O_LOC)
        m = {
            "xt": xt_bf16,
            "wt": np.ascontiguousarray(weight[sl].T),          # [I, O_LOC]
            "rs": np.ascontiguousarray(rs_full[sl].T),         # [G, O_LOC]
            "cs": np.ascontiguousarray(cs_full[sl].T),         # [G, O_LOC]
            "biasv": bias[sl].reshape(1, O_LOC).copy(),
        }
        if use_lora:
            # aw[i, o] = a*s[o, g(i)] + wl[o, i]
            aw_full = np.repeat(aw_base[sl], GS, axis=1) + wl[sl]  # [O_LOC, I]
            m["aw"] = np.ascontiguousarray(aw_full.T)              # [I, O_LOC]
        else:
            m["asb"] = np.ascontiguousarray(aw_base[sl].T)         # [G, O_LOC]
        in_maps.append(m)

    nc = _get_nc(use_lora)
    res = run_bass_kernel_spmd(
        nc, in_maps, core_ids=list(range(N_CORES)), trace=False
    )
    global LAST_RESULT
    LAST_RESULT = res

    y = np.concatenate([res.results[c]["y"] for c in range(N_CORES)], axis=1)
    return np.ascontiguousarray(y.reshape(B, S, O_FULL).astype(np.float32))


if __name__ == "__main__":
    rng = np.random.default_rng(0)
    x = rng.standard_normal((B, S, I_DIM), dtype=np.float32)
    weight = (rng.standard_normal((O_FULL, I_DIM), dtype=np.float32) * 0.02)
    bias = rng.uniform(-0.015, 0.015, O_FULL).astype(np.float32)
    sf = np.maximum(np.abs(weight.reshape(O_FULL, G, GS)).max(axis=2), EPS)
    u, s, vh = np.linalg.svd(sf, full_matrices=False)
    scale_A = (u[:, :4] * s[:4]).astype(np.float32)
    scale_B = vh[:4, :].astype(np.float32)
    lut = np.linspace(-1, 1, LUT_SIZE, dtype=np.float32)
    lora_A = rng.standard_normal((16, I_DIM), dtype=np.float32) * 0.02
    lora_B = np.zeros((O_FULL, 16), dtype=np.float32)
    y = kernel(x=x, weight=weight, bias=bias, scale_A=scale_A, scale_B=scale_B,
               lut=lut, lora_A=lora_A, lora_B=lora_B)
    print("kernel output:", y.shape, y.dtype)
